# revision 1
# baseline (speedup 1.0000x reference)
"""Trainium2 Bass kernel for nn_ConfidanceLoss.

reference semantics (see harness reference):
  occ   = (batchVolume == 1)                       [B, 32, 32, 32]
  pooled= 5x5x5 windowed max (zero-pad, stride 1)
  sub   = pooled sampled at cell centers 2,6,..,30 -> [B, 8, 8, 8] (x, y, z)
  iou   = transpose to (z, y, x) then flatten      -> [B, 512], j = z*64 + y*8 + x
  returns (confi [B,512] f32, iou [B,512] f32, in_use [B,512] i32)

Window for center 4i+2 is [4i, 4i+4] clipped to 31, so per axis:
  out[i] = max(V[4i], V[4i+1], V[4i+2], V[4i+3], V[4i+4 if 4i+4<=31])
Separable: pool A3 (inner), then A2, then A1, each as tensor_max trees over
strided views.  Batch items sit on the 128 SBUF partitions (128 per core,
8 cores = B=1024), so every reduction is along the free dimension.
"""

import sys

for _p in ("/opt/trn_rl_repo",):
    if _p not in sys.path:
        sys.path.insert(0, _p)

import numpy as np

import concourse.bass as bass  # noqa: F401  (registers types)
import concourse.tile as tile
from concourse import bacc, mybir
from concourse.bass_utils import run_bass_kernel_spmd

B = 1024
GRID = 32
P = 512
N_CORES = 8
ITEMS = B // N_CORES  # 128 batch items per core == 128 partitions
VOL = GRID * GRID * GRID  # 32768
N_CHUNKS = 4
PLANES = GRID // N_CHUNKS  # 8 A1-planes per chunk
CHUNK = PLANES * GRID * GRID  # 8192 elements per partition per chunk

_I32 = mybir.dt.int32
_F32 = mybir.dt.float32


def _build():
    nc = bacc.Bacc(
        "TRN2",
        target_bir_lowering=False,
        debug=False,
        num_devices=N_CORES,
    )
    vol = nc.dram_tensor("batchVolume", [ITEMS, VOL], _I32, kind="ExternalInput")
    confi = nc.dram_tensor("confi", [ITEMS, P], _F32, kind="ExternalInput")
    out_confi = nc.dram_tensor("out_confi", [ITEMS, P], _F32, kind="ExternalOutput")
    out_iou = nc.dram_tensor("out_iou", [ITEMS, P], _F32, kind="ExternalOutput")
    out_inuse = nc.dram_tensor("out_inuse", [ITEMS, P], _I32, kind="ExternalOutput")

    with tile.TileContext(nc) as tc:
        with (
            tc.tile_pool(name="vol", bufs=3) as vol_pool,
            tc.tile_pool(name="p1", bufs=2) as p1_pool,
            tc.tile_pool(name="p2", bufs=2) as p2_pool,
            tc.tile_pool(name="misc", bufs=1) as misc_pool,
        ):
            # confi passthrough (tiny)
            cbuf = misc_pool.tile([ITEMS, P], _F32, tag="cbuf")
            nc.sync.dma_start(cbuf[:], confi.ap())
            nc.sync.dma_start(out_confi.ap(), cbuf[:])

            # I: per-item [a1=32, c2=8, c3=8] intermediate after A3+A2 pooling
            I = misc_pool.tile([ITEMS, GRID * 64], _I32, tag="interm")

            for c in range(N_CHUNKS):
                vc = vol_pool.tile([ITEMS, CHUNK], _I32, tag="vc")
                nc.sync.dma_start(vc[:], vol.ap()[:, CHUNK * c : CHUNK * (c + 1)])
                V = vc[:].rearrange(
                    "p (a1 a2 a3) -> p a1 a2 a3", a1=PLANES, a2=GRID, a3=GRID
                )
                # ---- pass 1: pool A3 -> [p, a1=8, a2=32, c3=8]
                tA = p1_pool.tile([ITEMS, PLANES * GRID * 8], _I32, tag="tA")
                tB = p1_pool.tile([ITEMS, PLANES * GRID * 8], _I32, tag="tB")
                A = tA[:].rearrange(
                    "p (a1 a2 c3) -> p a1 a2 c3", a1=PLANES, a2=GRID, c3=8
                )
                Bv = tB[:].rearrange(
                    "p (a1 a2 c3) -> p a1 a2 c3", a1=PLANES, a2=GRID, c3=8
                )
                nc.vector.tensor_max(A, V[:, :, :, 0::4], V[:, :, :, 1::4])
                # fold the 5th (clipped) window element into A before combining
                nc.vector.tensor_max(
                    A[:, :, :, 0:7], A[:, :, :, 0:7], V[:, :, :, 4::4]
                )
                nc.vector.tensor_max(Bv, V[:, :, :, 2::4], V[:, :, :, 3::4])
                nc.vector.tensor_max(A, A, Bv)
                # ---- pass 2: pool A2 -> I[:, c*8+a1, c2, c3]
                sA = p2_pool.tile([ITEMS, PLANES * 64], _I32, tag="sA")
                sB = p2_pool.tile([ITEMS, PLANES * 64], _I32, tag="sB")
                SA = sA[:].rearrange("p (a1 c2 c3) -> p a1 c2 c3", a1=PLANES, c2=8, c3=8)
                SB = sB[:].rearrange("p (a1 c2 c3) -> p a1 c2 c3", a1=PLANES, c2=8, c3=8)
                Ic = I[:, PLANES * 64 * c : PLANES * 64 * (c + 1)].rearrange(
                    "p (a1 c2 c3) -> p a1 c2 c3", a1=PLANES, c2=8, c3=8
                )
                nc.vector.tensor_max(SA, A[:, :, 0::4, :], A[:, :, 1::4, :])
                nc.vector.tensor_max(
                    SA[:, :, 0:7, :], SA[:, :, 0:7, :], A[:, :, 4::4, :]
                )
                nc.vector.tensor_max(SB, A[:, :, 2::4, :], A[:, :, 3::4, :])
                nc.vector.tensor_max(Ic, SA, SB)

            # ---- pass 3: pool A1 over I [p, a1=32, f=64] -> permuted f32 out
            IA = I[:].rearrange("p (a1 f) -> p a1 f", a1=GRID, f=64)
            t1 = misc_pool.tile([ITEMS, 512], _I32, tag="t1")
            t2 = misc_pool.tile([ITEMS, 512], _I32, tag="t2")
            T1 = t1[:].rearrange("p (c1 f) -> p c1 f", c1=8, f=64)
            T2 = t2[:].rearrange("p (c1 f) -> p c1 f", c1=8, f=64)
            nc.vector.tensor_max(T1, IA[:, 0::4, :], IA[:, 1::4, :])
            nc.vector.tensor_max(T1[:, 0:7, :], T1[:, 0:7, :], IA[:, 4::4, :])
            nc.vector.tensor_max(T2, IA[:, 2::4, :], IA[:, 3::4, :])
            # final combine writes the (z, y, x)-flattened f32 output directly:
            # T* hold [c1=x, c2=y, c3=z]; out j = z*64 + y*8 + x
            iou_sb = misc_pool.tile([ITEMS, P], _F32, tag="iou")
            PV = iou_sb[:].rearrange("p (c3 c2 c1) -> p c1 c2 c3", c1=8, c2=8, c3=8)
            nc.vector.tensor_max(PV, T1, T2)
            inuse_sb = misc_pool.tile([ITEMS, P], _I32, tag="inuse")
            nc.vector.tensor_copy(inuse_sb[:], iou_sb[:])

            nc.sync.dma_start(out_iou.ap(), iou_sb[:])
            nc.sync.dma_start(out_inuse.ap(), inuse_sb[:])

    nc.compile()
    return nc


_NC_CACHE = None


def _get_nc():
    global _NC_CACHE
    if _NC_CACHE is None:
        _NC_CACHE = _build()
    return _NC_CACHE


def _make_in_maps(confi_rlt, batchVolume):
    confi = np.ascontiguousarray(
        confi_rlt.reshape(B, P).astype(np.float32, copy=False)
    )
    vol = np.ascontiguousarray(
        batchVolume.reshape(B, VOL).astype(np.int32, copy=False)
    )
    in_maps = []
    for c in range(N_CORES):
        sl = slice(ITEMS * c, ITEMS * (c + 1))
        in_maps.append(
            {
                "batchVolume": np.ascontiguousarray(vol[sl]),
                "confi": np.ascontiguousarray(confi[sl]),
            }
        )
    return in_maps


def _run(confi_rlt, batchVolume, trace=False, **spmd_kwargs):
    nc = _get_nc()
    res = run_bass_kernel_spmd(
        nc,
        _make_in_maps(confi_rlt, batchVolume),
        core_ids=list(range(N_CORES)),
        trace=trace,
        **spmd_kwargs,
    )
    confi_full = np.concatenate([r["out_confi"] for r in res.results], axis=0)
    iou_full = np.concatenate([r["out_iou"] for r in res.results], axis=0)
    inuse_full = np.concatenate([r["out_inuse"] for r in res.results], axis=0)
    return (confi_full, iou_full, inuse_full), res


def kernel(shape_rlt, trans_rlt, quat_rlt, confi_rlt, batchVolume):
    out, _ = _run(confi_rlt, batchVolume)
    return out


# revision 2
# speedup vs baseline: 1.2976x; 1.2976x over previous
"""Trainium2 Bass kernel for nn_ConfidanceLoss.

reference semantics (see harness reference):
  occ   = (batchVolume == 1)                       [B, 32, 32, 32]
  pooled= 5x5x5 windowed max (zero-pad, stride 1)
  sub   = pooled sampled at cell centers 2,6,..,30 -> [B, 8, 8, 8] (x, y, z)
  iou   = transpose to (z, y, x) then flatten      -> [B, 512], j = z*64 + y*8 + x
  returns (confi [B,512] f32, iou [B,512] f32, in_use [B,512] i32)

Window for center 4i+2 is [4i, 4i+4] clipped to 31, so per axis:
  out[i] = max(V[4i], V[4i+1], V[4i+2], V[4i+3], V[4i+4 if 4i+4<=31])

Separable max-pool, 128 batch items per core on the 128 SBUF partitions
(8 cores x 128 = B=1024); all reductions run along the free dimension.
Pass order is chosen for DVE read contiguity (strided inner reads run
~3x slower than contiguous runs):
  1. pool A2 (middle axis): reads contiguous 32-elem a3 rows   [a1,32,32]->[a1,8,32]
  2. pool A1 (outer axis):  reads contiguous 256-elem planes,  incremental per chunk
  3. pool A3 (inner axis):  stride-4 reads but on 16x-reduced data
The volume streams in as 8 chunks of 4 A1-planes (2 MiB DMAs) on the
sync (SP) HWDGE ring; confi passthrough + outputs use the scalar (ACT)
ring so they never block volume loads.
"""

import sys

for _p in ("/opt/trn_rl_repo",):
    if _p not in sys.path:
        sys.path.insert(0, _p)

import numpy as np

import concourse.bass as bass  # noqa: F401  (registers types)
import concourse.tile as tile
from concourse import bacc, mybir
from concourse.bass_utils import run_bass_kernel_spmd

B = 1024
GRID = 32
P = 512
N_CORES = 8
ITEMS = B // N_CORES  # 128 batch items per core == 128 partitions
VOL = GRID * GRID * GRID  # 32768
N_CHUNKS = 8
PLANES = GRID // N_CHUNKS  # 4 A1-planes per chunk
CHUNK = PLANES * GRID * GRID  # 4096 elements per partition per chunk

_I32 = mybir.dt.int32
_F32 = mybir.dt.float32


def _build():
    nc = bacc.Bacc(
        "TRN2",
        target_bir_lowering=False,
        debug=False,
        num_devices=N_CORES,
    )
    vol = nc.dram_tensor("batchVolume", [ITEMS, VOL], _I32, kind="ExternalInput")
    confi = nc.dram_tensor("confi", [ITEMS, P], _F32, kind="ExternalInput")
    out_confi = nc.dram_tensor("out_confi", [ITEMS, P], _F32, kind="ExternalOutput")
    out_iou = nc.dram_tensor("out_iou", [ITEMS, P], _F32, kind="ExternalOutput")
    out_inuse = nc.dram_tensor("out_inuse", [ITEMS, P], _I32, kind="ExternalOutput")

    with tile.TileContext(nc) as tc:
        with (
            tc.tile_pool(name="vol", bufs=7) as vol_pool,
            tc.tile_pool(name="tmp", bufs=2) as tmp_pool,
            tc.tile_pool(name="misc", bufs=1) as misc_pool,
        ):
            # confi passthrough on the ACT ring (tiny, off the volume path)
            cbuf = misc_pool.tile([ITEMS, P], _F32, tag="cbuf")
            nc.scalar.dma_start(cbuf[:], confi.ap())
            nc.scalar.dma_start(out_confi.ap(), cbuf[:])

            # after A2-pool: I [a1=32, c2=8, a3=32] per item
            I = misc_pool.tile([ITEMS, GRID * 8 * GRID], _I32, tag="interm")
            # after A1-pool: Pp [c1=8, c2=8, a3=32]
            Pp = misc_pool.tile([ITEMS, 8 * 8 * GRID], _I32, tag="ppool")
            PpV = Pp[:].rearrange("p (c1 f) -> p c1 f", c1=8, f=256)

            for c in range(N_CHUNKS):
                vc = vol_pool.tile([ITEMS, CHUNK], _I32, tag="vc")
                nc.sync.dma_start(vc[:], vol.ap()[:, CHUNK * c : CHUNK * (c + 1)])
                V = vc[:].rearrange(
                    "p (a1 a2 a3) -> p a1 a2 a3", a1=PLANES, a2=GRID, a3=GRID
                )
                # ---- pass 1: pool A2 -> I[:, 4c+a1, c2, a3] (contiguous reads)
                tA = tmp_pool.tile([ITEMS, PLANES * 8 * GRID], _I32, tag="tA")
                tB = tmp_pool.tile([ITEMS, PLANES * 8 * GRID], _I32, tag="tB")
                A = tA[:].rearrange(
                    "p (a1 c2 a3) -> p a1 c2 a3", a1=PLANES, c2=8, a3=GRID
                )
                Bv = tB[:].rearrange(
                    "p (a1 c2 a3) -> p a1 c2 a3", a1=PLANES, c2=8, a3=GRID
                )
                nc.vector.tensor_max(A, V[:, :, 0::4, :], V[:, :, 1::4, :])
                nc.vector.tensor_max(
                    A[:, :, 0:7, :], A[:, :, 0:7, :], V[:, :, 4::4, :]
                )
                nc.vector.tensor_max(Bv, V[:, :, 2::4, :], V[:, :, 3::4, :])
                Ic = I[:, PLANES * 256 * c : PLANES * 256 * (c + 1)]
                nc.vector.tensor_max(Ic, tA[:], tB[:])  # flat contiguous

                # ---- pass 2 (incremental): pool A1 within this chunk
                # chunk c == planes [4c, 4c+3] == window c1=c minus its 5th plane
                IcV = Ic.rearrange("p (a1 f) -> p a1 f", a1=PLANES, f=256)
                m = tmp_pool.tile([ITEMS, 2 * 256], _I32, tag="m")
                mV = m[:].rearrange("p (h f) -> p h f", h=2, f=256)
                nc.vector.tensor_max(mV, IcV[:, 0::2, :], IcV[:, 1::2, :])
                nc.vector.tensor_max(PpV[:, c : c + 1, :], mV[:, 0:1, :], mV[:, 1:2, :])
                if c > 0:
                    # fold plane 4c (this chunk's first) into window c1=c-1
                    nc.vector.tensor_max(
                        PpV[:, c - 1 : c, :], PpV[:, c - 1 : c, :], IcV[:, 0:1, :]
                    )

            # ---- pass 3: pool A3 (stride-4 reads on 16x-reduced data)
            PQ = Pp[:].rearrange("p (c1 c2 a3) -> p c1 c2 a3", c1=8, c2=8, a3=GRID)
            s1 = misc_pool.tile([ITEMS, P], _I32, tag="s1")
            s2 = misc_pool.tile([ITEMS, P], _I32, tag="s2")
            S1 = s1[:].rearrange("p (c1 c2 c3) -> p c1 c2 c3", c1=8, c2=8, c3=8)
            S2 = s2[:].rearrange("p (c1 c2 c3) -> p c1 c2 c3", c1=8, c2=8, c3=8)
            nc.vector.tensor_max(S1, PQ[:, :, :, 0::4], PQ[:, :, :, 1::4])
            nc.vector.tensor_max(
                S1[:, :, :, 0:7], S1[:, :, :, 0:7], PQ[:, :, :, 4::4]
            )
            nc.vector.tensor_max(S2, PQ[:, :, :, 2::4], PQ[:, :, :, 3::4])
            # final combine writes the (z, y, x)-flattened f32 output directly:
            # S* hold [c1=x, c2=y, c3=z]; out j = z*64 + y*8 + x
            iou_sb = misc_pool.tile([ITEMS, P], _F32, tag="iou")
            PV = iou_sb[:].rearrange("p (c3 c2 c1) -> p c1 c2 c3", c1=8, c2=8, c3=8)
            nc.vector.tensor_max(PV, S1, S2)
            inuse_sb = misc_pool.tile([ITEMS, P], _I32, tag="inuse")
            nc.vector.tensor_copy(inuse_sb[:], iou_sb[:])

            nc.scalar.dma_start(out_iou.ap(), iou_sb[:])
            nc.scalar.dma_start(out_inuse.ap(), inuse_sb[:])

    nc.compile()
    return nc


_NC_CACHE = None


def _get_nc():
    global _NC_CACHE
    if _NC_CACHE is None:
        _NC_CACHE = _build()
    return _NC_CACHE


def _make_in_maps(confi_rlt, batchVolume):
    confi = np.ascontiguousarray(
        confi_rlt.reshape(B, P).astype(np.float32, copy=False)
    )
    vol = np.ascontiguousarray(
        batchVolume.reshape(B, VOL).astype(np.int32, copy=False)
    )
    in_maps = []
    for c in range(N_CORES):
        sl = slice(ITEMS * c, ITEMS * (c + 1))
        in_maps.append(
            {
                "batchVolume": np.ascontiguousarray(vol[sl]),
                "confi": np.ascontiguousarray(confi[sl]),
            }
        )
    return in_maps


def _run(confi_rlt, batchVolume, trace=False, **spmd_kwargs):
    nc = _get_nc()
    res = run_bass_kernel_spmd(
        nc,
        _make_in_maps(confi_rlt, batchVolume),
        core_ids=list(range(N_CORES)),
        trace=trace,
        **spmd_kwargs,
    )
    confi_full = np.concatenate([r["out_confi"] for r in res.results], axis=0)
    iou_full = np.concatenate([r["out_iou"] for r in res.results], axis=0)
    inuse_full = np.concatenate([r["out_inuse"] for r in res.results], axis=0)
    return (confi_full, iou_full, inuse_full), res


def kernel(shape_rlt, trans_rlt, quat_rlt, confi_rlt, batchVolume):
    out, _ = _run(confi_rlt, batchVolume)
    return out


# revision 4
# speedup vs baseline: 1.3009x; 1.0025x over previous
"""Trainium2 Bass kernel for nn_ConfidanceLoss.

reference semantics (see harness reference):
  occ   = (batchVolume == 1)                       [B, 32, 32, 32]
  pooled= 5x5x5 windowed max (zero-pad, stride 1)
  sub   = pooled sampled at cell centers 2,6,..,30 -> [B, 8, 8, 8] (x, y, z)
  iou   = transpose to (z, y, x) then flatten      -> [B, 512], j = z*64 + y*8 + x
  returns (confi [B,512] f32, iou [B,512] f32, in_use [B,512] i32)

Window for center 4i+2 is [4i, 4i+4] clipped to 31, so per axis:
  out[i] = max(V[4i], V[4i+1], V[4i+2], V[4i+3], V[4i+4 if 4i+4<=31])

Separable max-pool, 128 batch items per core on the 128 SBUF partitions
(8 cores x 128 = B=1024); all reductions run along the free dimension.
Pass order is chosen for DVE read contiguity (inner-strided reads run
~3x slower than contiguous runs):
  1. pool A2 (middle axis): reads contiguous 32-elem a3 rows
  2. pool A1 (outer axis):  incremental per chunk, contiguous 256-elem planes
  3. pool A3 (inner axis):  stride-4 reads but on 16x-reduced data
The volume streams in as A1-plane chunks (first two small so DVE starts
early) on the sync (SP) HWDGE ring; confi passthrough + most outputs use
the scalar (ACT) ring so they never block volume loads.
"""

import sys

for _p in ("/opt/trn_rl_repo",):
    if _p not in sys.path:
        sys.path.insert(0, _p)

import numpy as np

import concourse.bass as bass  # noqa: F401  (registers types)
import concourse.tile as tile
from concourse import bacc, mybir
from concourse.bass_utils import run_bass_kernel_spmd

B = 1024
GRID = 32
P = 512
N_CORES = 8
ITEMS = B // N_CORES  # 128 batch items per core == 128 partitions
VOL = GRID * GRID * GRID  # 32768
ROW = GRID * GRID  # elems per A1-plane per item
CHUNK_PLANES = [2, 2] + [4] * 7  # sums to 32

_I32 = mybir.dt.int32
_F32 = mybir.dt.float32


def _build():
    nc = bacc.Bacc(
        "TRN2",
        target_bir_lowering=False,
        debug=False,
        num_devices=N_CORES,
    )
    vol = nc.dram_tensor("batchVolume", [ITEMS, VOL], _I32, kind="ExternalInput")
    confi = nc.dram_tensor("confi", [ITEMS, P], _F32, kind="ExternalInput")
    out_confi = nc.dram_tensor("out_confi", [ITEMS, P], _F32, kind="ExternalOutput")
    out_iou = nc.dram_tensor("out_iou", [ITEMS, P], _F32, kind="ExternalOutput")
    out_inuse = nc.dram_tensor("out_inuse", [ITEMS, P], _I32, kind="ExternalOutput")

    with tile.TileContext(nc) as tc:
        with (
            tc.tile_pool(name="vol", bufs=4) as vol_pool,
            tc.tile_pool(name="tmp", bufs=2) as tmp_pool,
            tc.tile_pool(name="misc", bufs=1) as misc_pool,
        ):
            # confi passthrough on the ACT ring (tiny, off the volume path)
            cbuf = misc_pool.tile([ITEMS, P], _F32, tag="cbuf")
            nc.scalar.dma_start(cbuf[:], confi.ap())
            nc.scalar.dma_start(out_confi.ap(), cbuf[:])

            # after A2-pool: I [a1=32, c2=8, a3=32] per item
            I = misc_pool.tile([ITEMS, GRID * 8 * GRID], _I32, tag="interm")
            # after A1-pool: Pp [c1=8, c2=8, a3=32]
            Pp = misc_pool.tile([ITEMS, 8 * 8 * GRID], _I32, tag="ppool")
            PpV = Pp[:].rearrange("p (c1 f) -> p c1 f", c1=8, f=256)
            PQ = Pp[:].rearrange("p (c1 c2 a3) -> p c1 c2 a3", c1=8, c2=8, a3=GRID)

            # A3-pool + output writes for a half (c1 range [w0, w1))
            s1 = misc_pool.tile([ITEMS, P], _I32, tag="s1")
            s2 = misc_pool.tile([ITEMS, P], _I32, tag="s2")
            S1 = s1[:].rearrange("p (c1 c2 c3) -> p c1 c2 c3", c1=8, c2=8, c3=8)
            S2 = s2[:].rearrange("p (c1 c2 c3) -> p c1 c2 c3", c1=8, c2=8, c3=8)
            iou_sb = misc_pool.tile([ITEMS, P], _F32, tag="iou")
            inuse_sb = misc_pool.tile([ITEMS, P], _I32, tag="inuse")
            # S* hold [c1=x, c2=y, c3=z]; out j = z*64 + y*8 + x
            PV = iou_sb[:].rearrange("p (c3 c2 c1) -> p c1 c2 c3", c1=8, c2=8, c3=8)
            PU = inuse_sb[:].rearrange("p (c3 c2 c1) -> p c1 c2 c3", c1=8, c2=8, c3=8)

            def pass_a3(w0, w1):
                q = PQ[:, w0:w1]
                a1, b1 = S1[:, w0:w1], S2[:, w0:w1]
                nc.vector.tensor_max(a1, q[:, :, :, 0::4], q[:, :, :, 1::4])
                nc.vector.tensor_max(a1[:, :, :, 0:7], a1[:, :, :, 0:7], q[:, :, :, 4::4])
                nc.vector.tensor_max(b1, q[:, :, :, 2::4], q[:, :, :, 3::4])
                nc.vector.tensor_max(PV[:, w0:w1], a1, b1)
                nc.vector.tensor_max(PU[:, w0:w1], a1, b1)

            plane0 = 0
            for c, planes in enumerate(CHUNK_PLANES):
                n = planes * ROW
                vc = vol_pool.tile([ITEMS, n], _I32, tag="vc")
                off = plane0 * ROW
                nc.sync.dma_start(vc[:], vol.ap()[:, off : off + n])
                V = vc[:].rearrange(
                    "p (a1 a2 a3) -> p a1 a2 a3", a1=planes, a2=GRID, a3=GRID
                )
                # ---- pass 1: pool A2 -> I planes [plane0, plane0+planes)
                tn = planes * 8 * GRID
                tB = tmp_pool.tile([ITEMS, tn], _I32, tag="tB")
                Bv = tB[:].rearrange(
                    "p (a1 c2 a3) -> p a1 c2 a3", a1=planes, c2=8, a3=GRID
                )
                Ic = I[:, 256 * plane0 : 256 * (plane0 + planes)]
                A = Ic.rearrange(
                    "p (a1 c2 a3) -> p a1 c2 a3", a1=planes, c2=8, a3=GRID
                )
                nc.vector.tensor_max(A, V[:, :, 0::4, :], V[:, :, 1::4, :])
                nc.vector.tensor_max(
                    A[:, :, 0:7, :], A[:, :, 0:7, :], V[:, :, 4::4, :]
                )
                nc.vector.tensor_max(Bv, V[:, :, 2::4, :], V[:, :, 3::4, :])
                nc.vector.tensor_max(Ic, Ic, tB[:])  # in-place flat combine
                IcV = Ic.rearrange("p (a1 f) -> p a1 f", a1=planes, f=256)

                # ---- pass 2 (incremental): fold these planes into A1 windows
                if c == 0:  # planes 0,1 -> start window 0
                    nc.vector.tensor_max(PpV[:, 0:1, :], IcV[:, 0:1, :], IcV[:, 1:2, :])
                elif c == 1:  # planes 2,3 -> finish window 0 (sans 5th plane)
                    nc.vector.tensor_max(
                        PpV[:, 0:1, :], PpV[:, 0:1, :], IcV[:, 0:1, :]
                    )
                    nc.vector.tensor_max(
                        PpV[:, 0:1, :], PpV[:, 0:1, :], IcV[:, 1:2, :]
                    )
                else:  # planes [4w, 4w+3] for window w = c-1
                    w = c - 1
                    m = tmp_pool.tile([ITEMS, 2 * 256], _I32, tag="m")
                    mV = m[:].rearrange("p (h f) -> p h f", h=2, f=256)
                    nc.vector.tensor_max(mV, IcV[:, 0::2, :], IcV[:, 1::2, :])
                    nc.vector.tensor_max(
                        PpV[:, w : w + 1, :], mV[:, 0:1, :], mV[:, 1:2, :]
                    )
                    # this chunk's first plane (4w) is window w-1's 5th plane
                    nc.vector.tensor_max(
                        PpV[:, w - 1 : w, :], PpV[:, w - 1 : w, :], IcV[:, 0:1, :]
                    )
                    if w == 4:  # windows 0..3 final -> first A3 half
                        pass_a3(0, 4)
                plane0 += planes

            pass_a3(4, 8)

            nc.sync.dma_start(out_iou.ap(), iou_sb[:])
            nc.scalar.dma_start(out_inuse.ap(), inuse_sb[:])

    nc.compile()
    return nc


_NC_CACHE = None


def _get_nc():
    global _NC_CACHE
    if _NC_CACHE is None:
        _NC_CACHE = _build()
    return _NC_CACHE


def _make_in_maps(confi_rlt, batchVolume):
    confi = np.ascontiguousarray(
        confi_rlt.reshape(B, P).astype(np.float32, copy=False)
    )
    vol = np.ascontiguousarray(
        batchVolume.reshape(B, VOL).astype(np.int32, copy=False)
    )
    in_maps = []
    for c in range(N_CORES):
        sl = slice(ITEMS * c, ITEMS * (c + 1))
        in_maps.append(
            {
                "batchVolume": np.ascontiguousarray(vol[sl]),
                "confi": np.ascontiguousarray(confi[sl]),
            }
        )
    return in_maps


def _run(confi_rlt, batchVolume, trace=False, **spmd_kwargs):
    nc = _get_nc()
    res = run_bass_kernel_spmd(
        nc,
        _make_in_maps(confi_rlt, batchVolume),
        core_ids=list(range(N_CORES)),
        trace=trace,
        **spmd_kwargs,
    )
    confi_full = np.concatenate([r["out_confi"] for r in res.results], axis=0)
    iou_full = np.concatenate([r["out_iou"] for r in res.results], axis=0)
    inuse_full = np.concatenate([r["out_inuse"] for r in res.results], axis=0)
    return (confi_full, iou_full, inuse_full), res


def kernel(shape_rlt, trans_rlt, quat_rlt, confi_rlt, batchVolume):
    out, _ = _run(confi_rlt, batchVolume)
    return out


# revision 5
# speedup vs baseline: 1.3048x; 1.0030x over previous
"""Trainium2 Bass kernel for nn_ConfidanceLoss.

reference semantics (see harness reference):
  occ   = (batchVolume == 1)                       [B, 32, 32, 32]
  pooled= 5x5x5 windowed max (zero-pad, stride 1)
  sub   = pooled sampled at cell centers 2,6,..,30 -> [B, 8, 8, 8] (x, y, z)
  iou   = transpose to (z, y, x) then flatten      -> [B, 512], j = z*64 + y*8 + x
  returns (confi [B,512] f32, iou [B,512] f32, in_use [B,512] i32)

Window for center 4i+2 is [4i, 4i+4] clipped to 31, so per axis:
  out[i] = max(V[4i], V[4i+1], V[4i+2], V[4i+3], V[4i+4 if 4i+4<=31])

Separable max-pool, 128 batch items per core on the 128 SBUF partitions
(8 cores x 128 = B=1024); all reductions run along the free dimension.
Pass order is chosen for DVE read contiguity (inner-strided reads run
~3x slower than contiguous runs):
  1. pool A2 (middle axis): reads contiguous 32-elem a3 rows
  2. pool A1 (outer axis):  incremental per chunk, contiguous 256-elem planes
  3. pool A3 (inner axis):  stride-4 reads but on 16x-reduced data
The volume streams in as A1-plane chunks (first two small so DVE starts
early) on the sync (SP) HWDGE ring; confi passthrough + most outputs use
the scalar (ACT) ring so they never block volume loads.
"""

import sys

for _p in ("/opt/trn_rl_repo",):
    if _p not in sys.path:
        sys.path.insert(0, _p)

import numpy as np

import concourse.bass as bass  # noqa: F401  (registers types)
import concourse.tile as tile
from concourse import bacc, mybir
from concourse.bass_utils import run_bass_kernel_spmd

B = 1024
GRID = 32
P = 512
N_CORES = 8
ITEMS = B // N_CORES  # 128 batch items per core == 128 partitions
VOL = GRID * GRID * GRID  # 32768
ROW = GRID * GRID  # elems per A1-plane per item
CHUNK_PLANES = [2, 2] + [4] * 7  # sums to 32

_I32 = mybir.dt.int32
_F32 = mybir.dt.float32
_BF16 = mybir.dt.bfloat16


def _build():
    nc = bacc.Bacc(
        "TRN2",
        target_bir_lowering=False,
        debug=False,
        num_devices=N_CORES,
    )
    vol = nc.dram_tensor("batchVolume", [ITEMS, VOL], _I32, kind="ExternalInput")
    confi = nc.dram_tensor("confi", [ITEMS, P], _F32, kind="ExternalInput")
    out_confi = nc.dram_tensor("out_confi", [ITEMS, P], _F32, kind="ExternalOutput")
    out_iou = nc.dram_tensor("out_iou", [ITEMS, P], _F32, kind="ExternalOutput")
    out_inuse = nc.dram_tensor("out_inuse", [ITEMS, P], _I32, kind="ExternalOutput")

    with tile.TileContext(nc) as tc:
        with (
            tc.tile_pool(name="vol", bufs=8) as vol_pool,
            tc.tile_pool(name="tmp", bufs=2) as tmp_pool,
            tc.tile_pool(name="misc", bufs=1) as misc_pool,
        ):
            # confi passthrough on the ACT ring (tiny, off the volume path)
            cbuf = misc_pool.tile([ITEMS, P], _F32, tag="cbuf")
            nc.scalar.dma_start(cbuf[:], confi.ap())
            nc.scalar.dma_start(out_confi.ap(), cbuf[:])

            # after A2-pool: I [a1=32, c2=8, a3=32] per item
            I = misc_pool.tile([ITEMS, GRID * 8 * GRID], _BF16, tag="interm")
            # after A1-pool: Pp [c1=8, c2=8, a3=32]
            Pp = misc_pool.tile([ITEMS, 8 * 8 * GRID], _BF16, tag="ppool")
            PpV = Pp[:].rearrange("p (c1 f) -> p c1 f", c1=8, f=256)
            PQ = Pp[:].rearrange("p (c1 c2 a3) -> p c1 c2 a3", c1=8, c2=8, a3=GRID)

            # A3-pool + output writes for a half (c1 range [w0, w1))
            s1 = misc_pool.tile([ITEMS, P], _BF16, tag="s1")
            s2 = misc_pool.tile([ITEMS, P], _BF16, tag="s2")
            S1 = s1[:].rearrange("p (c1 c2 c3) -> p c1 c2 c3", c1=8, c2=8, c3=8)
            S2 = s2[:].rearrange("p (c1 c2 c3) -> p c1 c2 c3", c1=8, c2=8, c3=8)
            iou_sb = misc_pool.tile([ITEMS, P], _F32, tag="iou")
            inuse_sb = misc_pool.tile([ITEMS, P], _I32, tag="inuse")
            # S* hold [c1=x, c2=y, c3=z]; out j = z*64 + y*8 + x
            PV = iou_sb[:].rearrange("p (c3 c2 c1) -> p c1 c2 c3", c1=8, c2=8, c3=8)

            def pass_a3(w0, w1):
                q = PQ[:, w0:w1]
                a1, b1 = S1[:, w0:w1], S2[:, w0:w1]
                nc.vector.tensor_max(a1, q[:, :, :, 0::4], q[:, :, :, 1::4])
                nc.vector.tensor_max(a1[:, :, :, 0:7], a1[:, :, :, 0:7], q[:, :, :, 4::4])
                nc.vector.tensor_max(b1, q[:, :, :, 2::4], q[:, :, :, 3::4])
                nc.vector.tensor_max(PV[:, w0:w1], a1, b1)

            plane0 = 0
            for c, planes in enumerate(CHUNK_PLANES):
                n = planes * ROW
                vc = vol_pool.tile([ITEMS, n], _BF16, tag="vc")
                off = plane0 * ROW
                nc.gpsimd.dma_start(
                    vc[:], vol.ap()[:, off : off + n], max_dma_last_dim=2048
                )
                V = vc[:].rearrange(
                    "p (a1 a2 a3) -> p a1 a2 a3", a1=planes, a2=GRID, a3=GRID
                )
                # ---- pass 1: pool A2 -> I planes [plane0, plane0+planes)
                tn = planes * 8 * GRID
                tB = tmp_pool.tile([ITEMS, tn], _BF16, tag="tB")
                Bv = tB[:].rearrange(
                    "p (a1 c2 a3) -> p a1 c2 a3", a1=planes, c2=8, a3=GRID
                )
                Ic = I[:, 256 * plane0 : 256 * (plane0 + planes)]
                A = Ic.rearrange(
                    "p (a1 c2 a3) -> p a1 c2 a3", a1=planes, c2=8, a3=GRID
                )
                nc.vector.tensor_max(A, V[:, :, 0::4, :], V[:, :, 1::4, :])
                nc.vector.tensor_max(
                    A[:, :, 0:7, :], A[:, :, 0:7, :], V[:, :, 4::4, :]
                )
                nc.vector.tensor_max(Bv, V[:, :, 2::4, :], V[:, :, 3::4, :])
                nc.vector.tensor_max(Ic, Ic, tB[:])  # in-place flat combine
                IcV = Ic.rearrange("p (a1 f) -> p a1 f", a1=planes, f=256)

                # ---- pass 2 (incremental): fold these planes into A1 windows
                if c == 0:  # planes 0,1 -> start window 0
                    nc.vector.tensor_max(PpV[:, 0:1, :], IcV[:, 0:1, :], IcV[:, 1:2, :])
                elif c == 1:  # planes 2,3 -> finish window 0 (sans 5th plane)
                    nc.vector.tensor_max(
                        PpV[:, 0:1, :], PpV[:, 0:1, :], IcV[:, 0:1, :]
                    )
                    nc.vector.tensor_max(
                        PpV[:, 0:1, :], PpV[:, 0:1, :], IcV[:, 1:2, :]
                    )
                else:  # planes [4w, 4w+3] for window w = c-1
                    w = c - 1
                    m = tmp_pool.tile([ITEMS, 2 * 256], _BF16, tag="m")
                    mV = m[:].rearrange("p (h f) -> p h f", h=2, f=256)
                    nc.vector.tensor_max(mV, IcV[:, 0::2, :], IcV[:, 1::2, :])
                    nc.vector.tensor_max(
                        PpV[:, w : w + 1, :], mV[:, 0:1, :], mV[:, 1:2, :]
                    )
                    # this chunk's first plane (4w) is window w-1's 5th plane
                    nc.vector.tensor_max(
                        PpV[:, w - 1 : w, :], PpV[:, w - 1 : w, :], IcV[:, 0:1, :]
                    )
                    if w == 4:  # windows 0..3 final -> first A3 half
                        pass_a3(0, 4)
                plane0 += planes

            pass_a3(4, 8)
            nc.vector.tensor_copy(inuse_sb[:], iou_sb[:])

            nc.sync.dma_start(out_iou.ap(), iou_sb[:])
            nc.scalar.dma_start(out_inuse.ap(), inuse_sb[:])

    nc.compile()
    return nc


_NC_CACHE = None


def _get_nc():
    global _NC_CACHE
    if _NC_CACHE is None:
        _NC_CACHE = _build()
    return _NC_CACHE


def _make_in_maps(confi_rlt, batchVolume):
    confi = np.ascontiguousarray(
        confi_rlt.reshape(B, P).astype(np.float32, copy=False)
    )
    vol = np.ascontiguousarray(
        batchVolume.reshape(B, VOL).astype(np.int32, copy=False)
    )
    in_maps = []
    for c in range(N_CORES):
        sl = slice(ITEMS * c, ITEMS * (c + 1))
        in_maps.append(
            {
                "batchVolume": np.ascontiguousarray(vol[sl]),
                "confi": np.ascontiguousarray(confi[sl]),
            }
        )
    return in_maps


def _run(confi_rlt, batchVolume, trace=False, **spmd_kwargs):
    nc = _get_nc()
    res = run_bass_kernel_spmd(
        nc,
        _make_in_maps(confi_rlt, batchVolume),
        core_ids=list(range(N_CORES)),
        trace=trace,
        **spmd_kwargs,
    )
    confi_full = np.concatenate([r["out_confi"] for r in res.results], axis=0)
    iou_full = np.concatenate([r["out_iou"] for r in res.results], axis=0)
    inuse_full = np.concatenate([r["out_inuse"] for r in res.results], axis=0)
    return (confi_full, iou_full, inuse_full), res


def kernel(shape_rlt, trans_rlt, quat_rlt, confi_rlt, batchVolume):
    out, _ = _run(confi_rlt, batchVolume)
    return out
